# revision 1
# baseline (speedup 1.0000x reference)
"""Trainium2 Bass kernel for softmax-free attention:
    q = x @ Wq^T; k = x @ Wk^T; v = x @ Wv^T
    s = (q @ k^T) / sqrt(d); out = s @ v
  x: [4, 4096, 1024], W*: [1024, 1024], out: [4, 4096, 1024] (fp32)

Sharding: 8 cores; core c handles batch c//2, sequence-half c%2 (2048 query
rows). Each core projects q/k/v only for its OWN 2048 rows and spills k/v
into a cross-core-visible Shared-DRAM buffer (slot = own rank-in-pair via a
dynamic DMA offset). The pair partner reads both halves at local HBM
bandwidth — no bulk collective. Ordering across the pair is a tiny token
AllReduce (the token is DMA-sampled from the shared buffers, so it carries a
RAW dep on all spill writes); only the peer-slot reads wait on it, and they
start ~120us after it fires. The per-core x input is column-ROTATED on the
host (own half first); attention is permutation-invariant over m as long as
k and v use the same order.

Layout strategy: the PE contracts over the partition dim, so every operand is
arranged K-on-partitions via host-side pre-transposes (xT = x[b].T, W^T) and
chained matmuls that produce transposed outputs directly:
  qT[e,l] = sum_d WqT[d,e] xT[d,l]     (lhsT=WqT chunk, rhs=xT chunk)
  kT[e,m] = likewise
  v[m,d'] = sum_d xT[d,m] WvT[d,d']    (lhsT=xT chunk,  rhs=WvT chunk)
  sT[m,l] = sum_e kT[e,m] qT[e,l]      (lhsT=kT chunk,  rhs=qT chunk)
  out[l,d']= sum_m sT[m,l] v[m,d']     (lhsT=sT chunk,  rhs=v chunk)
The 1/sqrt(d) scale is folded into WqT on the host. All matmul inputs are
float32r (full PE rate at free-dim>=256, ~1e-4 rel err).

Phase A streams the own xT half once, producing kT + v (spilled to shared
DRAM) and qT (kept resident in SBUF). Phase B processes the 2048 query rows in two 1024-row
blocks, streaming kT/v back in 512-row m-chunks (4-matmul PSUM accumulation
groups keep the PE efficient) and accumulating out in SBUF via DVE adds.
"""

import sys
import types
from contextlib import ExitStack

import numpy as np

import concourse.bass as bass
import concourse.tile as tile
from concourse import bacc, mybir
from concourse.bass_utils import run_bass_kernel_spmd
from concourse.mybir import EngineType
from concourse.tile import add_dep_helper
from concourse.vector_clock import ScopedClock

# ---------------------------------------------------------------------------
# Environment shims
# ---------------------------------------------------------------------------


def _install_tile_drain_patch():
    """This toolchain's walrus caps sync waits at 1 per instruction, but
    TileContext's tail drain can carry several. Split the overflow onto
    preceding nops (same semantics: the issuing engine observes every sem
    before draining)."""
    if getattr(tile.TileContext, "_drain_patch_installed", False):
        return

    def _patched_drain_and_barrier(self, tick_clock, wait_clock):
        nc = self.nc
        collector = nc.sync.nop(hint="drain_wait_collector", nofuse=True)
        wait_clock.add_sem_waits(
            collector.ins, ScopedClock({None: tick_clock.global_clock})
        )
        waits = list(collector.ins.sync_info.on_wait or [])
        if len(waits) > 1:
            collector.ins.sync_info.on_wait = [waits[0]]
            for w in waits[1:]:
                nop = nc.sync.nop(hint="drain_wait_extra", nofuse=True)
                nop.ins.sync_info = mybir.SyncInfo(on_wait=[w], on_update=[])
        nc.sync.drain()

        nc.all_engine_barrier()
        assert self.sems is not None
        popped = nc._tile_sem_poison_stack.pop()
        assert popped is self._sem_poison
        nc.clear_and_free_semaphores(list(self.sems.allocated().values()))
        nc.all_engine_barrier()

    tile.TileContext._drain_and_barrier = _patched_drain_and_barrier
    tile.TileContext._drain_patch_installed = True


def _install_ntff_shim():
    """The image's antenv lacks axon_hooks, which silently degrades
    trace=True. Recreate the get/set pair and register the ctypes NTFF hook
    from trn_agent_boot (no-op if unavailable)."""
    if "antenv.axon_hooks" in sys.modules:
        return
    state = {"hook": None}

    def set_axon_ntff_profile_hook(h):
        state["hook"] = h

    def get_axon_ntff_profile_hook():
        return state["hook"]

    mod = types.ModuleType("antenv.axon_hooks")
    mod.set_axon_ntff_profile_hook = set_axon_ntff_profile_hook
    mod.get_axon_ntff_profile_hook = get_axon_ntff_profile_hook
    sys.modules["antenv.axon_hooks"] = mod
    try:
        import antenv

        antenv.axon_hooks = mod
        from trn_agent_boot.trn_boot import _ntff_profile_via_ctypes

        set_axon_ntff_profile_hook(
            _ntff_profile_via_ctypes("/opt/axon/libaxon_pjrt.so")
        )
    except Exception:
        pass


_install_tile_drain_patch()
_install_ntff_shim()

# ---------------------------------------------------------------------------
# Problem constants (hardcoded per the harness contract)
# ---------------------------------------------------------------------------

B, L, D = 4, 4096, 1024
N_CORES = 8
P = 128
LH = L // 2  # query rows per core
DC = D // P  # 8 contraction chunks of 128 over d/e
F32 = mybir.dt.float32
F32R = mybir.dt.float32r

ACHUNK = 512  # phase-A xT column chunk
BCHUNK = 512  # phase-B m-chunk (k/v rows)
PAIRS = [[2 * i, 2 * i + 1] for i in range(N_CORES // 2)]
BMS = BCHUNK // P
LBLOCK = 1024  # phase-B query-block rows


def build_nc():
    nc = bacc.Bacc("TRN2", target_bir_lowering=False, debug=False,
                   num_devices=N_CORES)
    xTr = nc.dram_tensor("xTr", [D, L], F32, kind="ExternalInput").ap()
    wqT = nc.dram_tensor("wqT", [D, D], F32, kind="ExternalInput").ap()
    wkT = nc.dram_tensor("wkT", [D, D], F32, kind="ExternalInput").ap()
    wvT = nc.dram_tensor("wvT", [D, D], F32, kind="ExternalInput").ap()
    out = nc.dram_tensor("out", [LH, D], F32, kind="ExternalOutput").ap()
    slots = nc.dram_tensor("slots", [1, 2], mybir.dt.uint32,
                           kind="ExternalInput").ap()
    kTsh = nc.dram_tensor("kTsh", [2, D, LH], F32R, addr_space="Shared").ap()
    vsh = nc.dram_tensor("vsh", [2, LH, D], F32R, addr_space="Shared").ap()
    tok = nc.dram_tensor("tok", [1, 2], F32).ap()
    tok2 = nc.dram_tensor("tok2", [1, 2], F32).ap()
    wu_sink = nc.dram_tensor("wu_sink", [P, ACHUNK], F32).ap()

    def chunked(ap):  # [K*, N] dram -> [P, K*/P, N] partition-major
        return ap.rearrange("(c p) n -> p c n", p=P)

    with tile.TileContext(nc) as tc, ExitStack() as octx:
        psum = octx.enter_context(tc.tile_pool(name="psum", bufs=8, space="PSUM"))
        qpool = octx.enter_context(tc.tile_pool(name="qpool", bufs=1))
        qsb = qpool.tile([P, DC, LH], F32R, tag="qsb")  # qT, SBUF-resident

        # HAM warmup: ~17us of junk matmuls while the first DMAs load, so the
        # PE clock gate is already at 8/8 when real work arrives
        with tc.tile_pool(name="wupool", bufs=1) as wupool:
            wut = wupool.tile([P, ACHUNK], F32R, tag="wut")
            nc.vector.memset(wut[:].bitcast(F32), 0.0)
            wuo = wupool.tile([P, ACHUNK], F32, tag="wuo")
            for g in range(20):
                wp = psum.tile([P, ACHUNK], F32, tag="ps", name=f"wu_{g}")
                for r in range(2):
                    nc.tensor.matmul(wp[:], wut[:, 0:P], wut[:],
                                     start=(r == 0), stop=(r == 1))
                if g == 19:
                    nc.vector.tensor_copy(wuo[:], wp[:])
            nc.sync.dma_start(wu_sink[:], wuo[:])

        # ---------------- Phase A: projections from one xT stream ----------
        # one interleaved loop (k, v, q per chunk — no intra-A transitions);
        # spills batched at 512KB so the sync ring's issue overhead fits the
        # per-chunk compute budget
        with ExitStack() as actx:
            wpool = actx.enter_context(tc.tile_pool(name="wpool", bufs=1))
            xpool = actx.enter_context(tc.tile_pool(name="xpool", bufs=2))
            stage = actx.enter_context(tc.tile_pool(name="stage", bufs=2))

            wq = wpool.tile([P, DC, D], F32R, tag="wq")
            wk = wpool.tile([P, DC, D], F32R, tag="wk")
            wv = wpool.tile([P, DC, D], F32R, tag="wv")
            # spread initial loads over rings: first MMs need wk + xc0 only,
            # and only their first c-chunks — split wk per chunk
            for c in range(DC):
                nc.sync.dma_start(wk[:, c], chunked(wkT).bitcast(F32R)[:, c])
            nc.gpsimd.dma_start(wv[:], chunked(wvT).bitcast(F32R))
            nc.gpsimd.dma_start(wq[:], chunked(wqT).bitcast(F32R))

            # rank-in-pair slot selectors for the shared spill buffers
            st_sl = stage.tile([1, 2], mybir.dt.uint32, tag="sl", bufs=1)
            nc.sync.dma_start(st_sl[:], slots[:])
            regs_o = nc.alloc_registers(
                engines=[EngineType.SP, EngineType.Activation])
            nc.regs_load(regs_o, st_sl[0:1, 0:1])
            svo = nc.snap(regs_o, donate=True)
            regs_p = nc.alloc_registers(
                engines=[EngineType.SP, EngineType.Activation])
            nc.regs_load(regs_p, st_sl[0:1, 1:2])
            svp = nc.snap(regs_p, donate=True)

            for j in range(LH // ACHUNK):
                xc = xpool.tile([P, DC, ACHUNK], F32R, tag="xc")
                cols = slice(j * ACHUNK, (j + 1) * ACHUNK)
                if j == 0:
                    # split per c-chunk: the first accumulation group starts
                    # after 768KB instead of 6MB of DMA
                    for c in range(DC):
                        nc.scalar.dma_start(
                            xc[:, c], chunked(xTr[:, cols]).bitcast(F32R)[:, c])
                else:
                    nc.scalar.dma_start(xc[:], chunked(xTr[:, cols]).bitcast(F32R))

                # kT chunk -> spill (four 512KB batches of 2 e-tiles)
                for eh in range(4):
                    kst = stage.tile([P, 2, ACHUNK], F32R, tag="st")
                    for ei in range(2):
                        e = eh * 2 + ei
                        pt = psum.tile([P, ACHUNK], F32, tag="ps")
                        for c in range(DC):
                            nc.tensor.matmul(
                                pt[:], wk[:, c, e * P:(e + 1) * P], xc[:, c],
                                start=(c == 0), stop=(c == DC - 1))
                        nc.vector.tensor_copy(kst[:, ei], pt[:])
                    nc.sync.dma_start(
                        kTsh[bass.ds(svo, 1), eh * 2 * P:(eh + 1) * 2 * P,
                             cols].rearrange("s (c p) n -> p (s c) n", p=P),
                        kst[:])

                # v chunk -> spill (four 512KB batches of 1 row-tile)
                for ms in range(ACHUNK // P):
                    row0 = j * ACHUNK + ms * P
                    vst = stage.tile([P, 1, D], F32R, tag="st", name=f"vst_{j}_{ms}")
                    for dh in range(D // ACHUNK):
                        pt = psum.tile([P, ACHUNK], F32, tag="ps")
                        dsl = slice(dh * ACHUNK, (dh + 1) * ACHUNK)
                        for c in range(DC):
                            nc.tensor.matmul(
                                pt[:], xc[:, c, ms * P:(ms + 1) * P],
                                wv[:, c, dsl],
                                start=(c == 0), stop=(c == DC - 1))
                        nc.vector.tensor_copy(vst[:, 0, dsl], pt[:])
                    nc.sync.dma_start(
                        vsh[bass.ds(svo, 1), row0:row0 + P, :].rearrange(
                            "s (c p) n -> p (s c) n", p=P),
                        vst[:])

                # qT chunk (first half of the rotated stream) -> resident
                if j < LH // ACHUNK:
                    for e in range(DC):
                        pt = psum.tile([P, ACHUNK], F32, tag="ps")
                        for c in range(DC):
                            nc.tensor.matmul(
                                pt[:], wq[:, c, e * P:(e + 1) * P], xc[:, c],
                                start=(c == 0), stop=(c == DC - 1))
                        nc.vector.tensor_copy(qsb[:, e, cols], pt[:])

            # pair barrier: the token is sampled from the shared buffers, so
            # its DMA carries a RAW dep on every spill write; the AllReduce
            # completes only when BOTH pair members' spills are durable
            tkt = stage.tile([1, 2], F32, tag="tkt", bufs=1)
            nc.sync.dma_start(tkt[0:1, 0:1], kTsh[bass.ds(svo, 1), 0:1, 0:1]
                              .rearrange("s c n -> c s n").bitcast(F32))
            nc.sync.dma_start(tkt[0:1, 1:2], vsh[bass.ds(svo, 1), 0:1, 0:1]
                              .rearrange("s c n -> c s n").bitcast(F32))
            nc.sync.dma_start(tok[:], tkt[:])
            pair_barrier = nc.gpsimd.collective_compute(
                "AllReduce", mybir.AluOpType.add, replica_groups=PAIRS,
                ins=[tok], outs=[tok2])

        # ---------------- Phase B: attention over m, single query block ----
        with ExitStack() as bctx:
            opool = bctx.enter_context(tc.tile_pool(name="opool", bufs=1))
            kpool = bctx.enter_context(tc.tile_pool(name="kpool", bufs=2))
            vpool = bctx.enter_context(tc.tile_pool(name="vpool", bufs=2))
            spool = bctx.enter_context(tc.tile_pool(name="spool", bufs=2))

            for lb in range(LH // LBLOCK):
                lsl0 = lb * LBLOCK
                ob = opool.tile([P, LBLOCK // P, D], F32, tag="ob")

                for j in range(L // BCHUNK):
                    # chunks 0-3: own half; 4-7: peer half (after the barrier)
                    own = j < LH // BCHUNK
                    sl = svo if own else svp
                    jj = j % (LH // BCHUNK)
                    msl = slice(jj * BCHUNK, (jj + 1) * BCHUNK)
                    kc = kpool.tile([P, DC, BCHUNK], F32R, tag="kc")
                    kld = nc.sync.dma_start(kc[:], kTsh[
                        bass.ds(sl, 1), :, msl].rearrange(
                        "s (c p) m -> p (s c) m", p=P))
                    vc = vpool.tile([P, BMS, D], F32R, tag="vc")
                    vld = nc.scalar.dma_start(vc[:], vsh[
                        bass.ds(sl, 1), msl, :].rearrange(
                        "s (c p) n -> p (s c) n", p=P))
                    if not own:
                        add_dep_helper(kld.ins, pair_barrier.ins,
                                       reason="peer kc after pair barrier")
                        add_dep_helper(vld.ins, pair_barrier.ins,
                                       reason="peer vc after pair barrier")

                    # sT chunk: [BCHUNK(m), LBLOCK(l)] as BMS tiles [P, LBLOCK]
                    sc = spool.tile([P, BMS, LBLOCK], F32R, tag="sc")
                    for ms in range(BMS):
                        for lh in range(LBLOCK // ACHUNK):
                            pt = psum.tile([P, ACHUNK], F32, tag="ps")
                            ls = slice(lh * ACHUNK, (lh + 1) * ACHUNK)
                            for e in range(DC):
                                nc.tensor.matmul(
                                    pt[:], kc[:, e, ms * P:(ms + 1) * P],
                                    qsb[:, e, lsl0 + lh * ACHUNK:
                                        lsl0 + (lh + 1) * ACHUNK],
                                    start=(e == 0), stop=(e == DC - 1))
                            nc.vector.tensor_copy(sc[:, ms, ls], pt[:])

                    # out += sT^T @ v, accumulated into ob
                    for lt in range(LBLOCK // P):
                        for dh in range(D // ACHUNK):
                            pt = psum.tile([P, ACHUNK], F32, tag="ps")
                            dsl = slice(dh * ACHUNK, (dh + 1) * ACHUNK)
                            for ms in range(BMS):
                                nc.tensor.matmul(
                                    pt[:], sc[:, ms, lt * P:(lt + 1) * P],
                                    vc[:, ms, dsl],
                                    start=(ms == 0), stop=(ms == BMS - 1))
                            if j == 0:
                                nc.vector.tensor_copy(ob[:, lt, dsl], pt[:])
                            else:
                                nc.vector.tensor_add(
                                    ob[:, lt, dsl], ob[:, lt, dsl], pt[:])

                # per-tile stores so the tail overlaps the last flush-adds
                for lt in range(LBLOCK // P):
                    row0 = lsl0 + lt * P
                    nc.sync.dma_start(out[row0:row0 + P, :], ob[:, lt])

    nc.compile()
    return nc


_NC_CACHE = {}


def _get_nc():
    if "nc" not in _NC_CACHE:
        _NC_CACHE["nc"] = build_nc()
    return _NC_CACHE["nc"]


def run(inputs, trace=False):
    """Run the kernel on all 8 cores. Returns (full_output, BassKernelResults)."""
    x = np.asarray(inputs["x"], dtype=np.float32)
    Wq = np.asarray(inputs["Wq"], dtype=np.float32)
    Wk = np.asarray(inputs["Wk"], dtype=np.float32)
    Wv = np.asarray(inputs["Wv"], dtype=np.float32)

    xT = np.ascontiguousarray(x.transpose(0, 2, 1))  # [B, D, L]
    inv_sqrt_d = np.float32(1.0 / np.sqrt(D))
    wqT = np.ascontiguousarray(Wq.T * inv_sqrt_d)
    wkT = np.ascontiguousarray(Wk.T)
    wvT = np.ascontiguousarray(Wv.T)

    in_maps = []
    for c in range(N_CORES):
        b, h = c // 2, c % 2
        # rotate columns so this core's own half comes first
        xtb = xT[b]
        xtr = np.concatenate(
            [xtb[:, h * LH:(h + 1) * LH], xtb[:, (1 - h) * LH:(2 - h) * LH]],
            axis=1)
        in_maps.append({
            "xTr": np.ascontiguousarray(xtr),
            "slots": np.array([[h, 1 - h]], dtype=np.uint32),
            "wqT": wqT, "wkT": wkT, "wvT": wvT,
        })

    nc = _get_nc()
    res = run_bass_kernel_spmd(nc, in_maps, list(range(N_CORES)), trace=trace)

    full = np.empty((B, L, D), dtype=np.float32)
    for c in range(N_CORES):
        b, h = c // 2, c % 2
        full[b, h * LH:(h + 1) * LH, :] = res.results[c]["out"]
    return full, res


def kernel(**inputs):
    full, _ = run(inputs, trace=False)
    return full



# revision 4
# speedup vs baseline: 2.7605x; 2.7605x over previous
"""Trainium2 Bass kernel for softmax-free attention:
    q = x @ Wq^T; k = x @ Wk^T; v = x @ Wv^T
    s = (q @ k^T) / sqrt(d); out = s @ v
  x: [4, 4096, 1024], W*: [1024, 1024], out: [4, 4096, 1024] (fp32)

No softmax => the whole map is linear and can be re-associated:
    out[b] = x[b] @ A[b],  A[b] = PT^T @ (G[b] @ Wv^T),
    G[b] = x[b]^T x[b],    PT = Wk^T (Wq/sqrt(d))   (host-folded weights)
This cuts device MACs ~4x vs the naive chain (projections + LxL scores).

Sharding: 8 cores; core c handles batch b=c//2, sequence-half h=c%2.  Both
pair members stream the FULL x[b] row-split: each computes the partial gram
G_part = x_half^T x_half over its OWN 2048 rows, spills the 4MB partial to a
cross-core-visible Shared-DRAM slot, and adds the peer's partial (read at
local HBM bandwidth) to form the full G.  Ordering across the pair is a tiny
token AllReduce per spill pass (the token is DMA-sampled from the shared
buffer, so it carries a RAW dep on the pass's spill writes); only the peer
reads wait on it.  After G, the chain V1 = G @ WvT (uses G's symmetry to
keep the contraction on partitions), A = PT^T @ V1 is duplicated across the
pair (cheap: 2 x 27us), and out_half = x_half @ A covers the core's own 2048
output rows.

Layout: the PE contracts over the partition dim; every matmul is arranged so
its output lands partition-major for the next stage:
  G[d,g]   = sum_l  x[l,d] x[l,g]      (lhsT=x chunk col-slice, rhs=x chunk)
  V1[d,d'] = sum_g  G[g,d] WvT[g,d']   (lhsT=G row-chunk == col-chunk, sym!)
  A[e,d']  = sum_d  PT[d,e] V1[d,d']   (lhsT=PT chunk, rhs=V1 chunk)
  out[l,d']= sum_e  xT[e,l] A[e,d']    (lhsT=xT_half chunk, rhs=A chunk)
All matmul inputs are float32r (full PE rate at free-dim>=256, ~1e-4 rel
err); PSUM accumulates fp32.  G's 2048-row partial runs as two passes of 8
resident PSUM banks (4 d-tiles x 2 g-halves x 16 l-chunks each); each pass
spills immediately so the pair exchange overlaps the second pass.
"""

import sys
import types
from contextlib import ExitStack

import numpy as np

import concourse.bass as bass
import concourse.tile as tile
from concourse import bacc, mybir
from concourse.bass_utils import run_bass_kernel_spmd
from concourse.mybir import EngineType
from concourse.tile import add_dep_helper
from concourse.vector_clock import ScopedClock

# ---------------------------------------------------------------------------
# Environment shims
# ---------------------------------------------------------------------------


def _install_tile_drain_patch():
    """This toolchain's walrus caps sync waits at 1 per instruction, but
    TileContext's tail drain can carry several. Split the overflow onto
    preceding nops (same semantics: the issuing engine observes every sem
    before draining)."""
    if getattr(tile.TileContext, "_drain_patch_installed", False):
        return

    def _patched_drain_and_barrier(self, tick_clock, wait_clock):
        nc = self.nc
        collector = nc.sync.nop(hint="drain_wait_collector", nofuse=True)
        wait_clock.add_sem_waits(
            collector.ins, ScopedClock({None: tick_clock.global_clock})
        )
        waits = list(collector.ins.sync_info.on_wait or [])
        if len(waits) > 1:
            collector.ins.sync_info.on_wait = [waits[0]]
            for w in waits[1:]:
                nop = nc.sync.nop(hint="drain_wait_extra", nofuse=True)
                nop.ins.sync_info = mybir.SyncInfo(on_wait=[w], on_update=[])
        nc.sync.drain()

        nc.all_engine_barrier()
        assert self.sems is not None
        popped = nc._tile_sem_poison_stack.pop()
        assert popped is self._sem_poison
        nc.clear_and_free_semaphores(list(self.sems.allocated().values()))
        nc.all_engine_barrier()

    tile.TileContext._drain_and_barrier = _patched_drain_and_barrier
    tile.TileContext._drain_patch_installed = True


def _install_ntff_shim():
    """The image's antenv lacks axon_hooks, which silently degrades
    trace=True. Recreate the get/set pair and register the ctypes NTFF hook
    from trn_agent_boot (no-op if unavailable)."""
    if "antenv.axon_hooks" in sys.modules:
        return
    state = {"hook": None}

    def set_axon_ntff_profile_hook(h):
        state["hook"] = h

    def get_axon_ntff_profile_hook():
        return state["hook"]

    mod = types.ModuleType("antenv.axon_hooks")
    mod.set_axon_ntff_profile_hook = set_axon_ntff_profile_hook
    mod.get_axon_ntff_profile_hook = get_axon_ntff_profile_hook
    sys.modules["antenv.axon_hooks"] = mod
    try:
        import antenv

        antenv.axon_hooks = mod
        from trn_agent_boot.trn_boot import _ntff_profile_via_ctypes

        set_axon_ntff_profile_hook(
            _ntff_profile_via_ctypes("/opt/axon/libaxon_pjrt.so")
        )
    except Exception:
        pass


_install_tile_drain_patch()
_install_ntff_shim()

# ---------------------------------------------------------------------------
# Problem constants (hardcoded per the harness contract)
# ---------------------------------------------------------------------------

B, L, D = 4, 4096, 1024
N_CORES = 8
P = 128
LH = L // 2  # rows per core
DC = D // P  # 8 chunks of 128 over d/e/g
F32 = mybir.dt.float32
F32R = mybir.dt.float32r
FREE = 512  # rhs free dim per matmul (one PSUM bank)

PAIRS = [[2 * i, 2 * i + 1] for i in range(N_CORES // 2)]
GPASS = 2          # G partial computed in 2 passes of 8 PSUM banks
LCH = LH // P      # 16 l-chunks of own half
LCHP = LCH // GPASS  # 8 l-chunks per pass


def build_nc():
    nc = bacc.Bacc("TRN2", target_bir_lowering=False, debug=False,
                   num_devices=N_CORES)
    xb = nc.dram_tensor("xb", [LH, D], F32, kind="ExternalInput").ap()
    xTh = nc.dram_tensor("xTh", [D, LH], F32, kind="ExternalInput").ap()
    wvT = nc.dram_tensor("wvT", [D, D], F32, kind="ExternalInput").ap()
    ptm = nc.dram_tensor("ptm", [D, D], F32, kind="ExternalInput").ap()
    out = nc.dram_tensor("out", [LH, D], F32, kind="ExternalOutput").ap()
    slots = nc.dram_tensor("slots", [1, 2], mybir.dt.uint32,
                           kind="ExternalInput").ap()
    Gsh = nc.dram_tensor("Gsh", [2, D, D], F32R, addr_space="Shared").ap()
    toks = [nc.dram_tensor(f"tok{p}", [1, 1], F32).ap() for p in range(GPASS)]
    toks2 = [nc.dram_tensor(f"tok2{p}", [1, 1], F32).ap()
             for p in range(GPASS)]
    wu_sink = nc.dram_tensor("wu_sink", [P, FREE], F32).ap()

    def chunked(ap):  # [K*, N] dram -> [P, K*/P, N] partition-major
        return ap.rearrange("(c p) n -> p c n", p=P)

    with tile.TileContext(nc) as tc, ExitStack() as octx:
        psum = octx.enter_context(tc.tile_pool(name="psum", bufs=8,
                                               space="PSUM"))
        gpool = octx.enter_context(tc.tile_pool(name="gpool", bufs=1))
        wpool = octx.enter_context(tc.tile_pool(name="wpool", bufs=1))
        mpool = octx.enter_context(tc.tile_pool(name="mpool", bufs=2))

        gsb = gpool.tile([P, DC, D], F32R, tag="gsb")  # full G after add
        wv = wpool.tile([P, DC, D], F32R, tag="wv")
        pt = wpool.tile([P, DC, D], F32R, tag="pt")
        nc.gpsimd.dma_start(wv[:], chunked(wvT).bitcast(F32R))
        nc.gpsimd.dma_start(pt[:], chunked(ptm).bitcast(F32R))

        # rank-in-pair slot selectors for the shared spill buffer
        st_sl = mpool.tile([1, 2], mybir.dt.uint32, tag="sl", bufs=1)
        nc.sync.dma_start(st_sl[:], slots[:])
        regs_o = nc.alloc_registers(
            engines=[EngineType.SP, EngineType.Activation])
        nc.regs_load(regs_o, st_sl[0:1, 0:1])
        svo = nc.snap(regs_o, donate=True)
        regs_p = nc.alloc_registers(
            engines=[EngineType.SP, EngineType.Activation])
        nc.regs_load(regs_p, st_sl[0:1, 1:2])
        svp = nc.snap(regs_p, donate=True)

        # HAM warmup: junk matmuls while the first DMAs load, so the PE
        # clock gate is already at 8/8 when real work arrives
        barriers = [None] * GPASS
        with ExitStack() as gctx:
            wupool = gctx.enter_context(tc.tile_pool(name="wupool", bufs=1))
            xpool = gctx.enter_context(tc.tile_pool(name="xpool", bufs=1))

            wut = wupool.tile([P, FREE], F32R, tag="wut")
            nc.vector.memset(wut[:].bitcast(F32), 0.0)
            wuo = wupool.tile([P, FREE], F32, tag="wuo")
            for g in range(20):
                wp = psum.tile([P, FREE], F32, tag="ps", name=f"wu_{g}")
                for r in range(2):
                    nc.tensor.matmul(wp[:], wut[:, 0:P], wut[:],
                                     start=(r == 0), stop=(r == 1))
                if g == 19:
                    nc.vector.tensor_copy(wuo[:], wp[:])
            nc.sync.dma_start(wu_sink[:], wuo[:])

            # ------------- Phase G: partial gram over own 2048 rows --------
            xsb = xpool.tile([P, LCH, D], F32R, tag="xsb")
            for c in range(LCH // 2):  # 1MB chunks so lc=0 is ready early
                nc.scalar.dma_start(
                    xsb[:, 2 * c:2 * c + 2],
                    chunked(xb).bitcast(F32R)[:, 2 * c:2 * c + 2])

            for gp in range(GPASS):
                dts = range(gp * DC // GPASS, (gp + 1) * DC // GPASS)
                gps = {}
                for dt in dts:
                    for gh in range(2):
                        gps[dt, gh] = psum.tile([P, FREE], F32, tag="ps",
                                                name=f"g_{dt}_{gh}")
                for lc in range(LCH):
                    for dt in dts:
                        for gh in range(2):
                            nc.tensor.matmul(
                                gps[dt, gh][:],
                                xsb[:, lc, dt * P:(dt + 1) * P],
                                xsb[:, lc, gh * FREE:(gh + 1) * FREE],
                                start=(lc == 0),
                                stop=(lc == LCH - 1))
                for dt in dts:
                    for gh in range(2):
                        nc.vector.tensor_copy(
                            gsb[:, dt, gh * FREE:(gh + 1) * FREE],
                            gps[dt, gh][:])
                # spill this pass's 2MB so the exchange overlaps pass 2
                d0 = gp * (D // GPASS)
                nc.sync.dma_start(
                    Gsh[bass.ds(svo, 1), d0:d0 + D // GPASS, :].rearrange(
                        "s (c p) n -> p (s c) n", p=P),
                    gsb[:, gp * DC // GPASS:(gp + 1) * DC // GPASS])
                # pair barrier for this pass: token sampled from the shared
                # buffer carries a RAW dep on the spill; AllReduce completes
                # only when BOTH pair members' pass-gp spills are durable
                tkt = mpool.tile([1, 1], F32, tag=f"tkt{gp}", bufs=1)
                nc.sync.dma_start(
                    tkt[0:1, 0:1],
                    Gsh[bass.ds(svo, 1), d0:d0 + 1, 0:1].rearrange(
                        "s c n -> c s n").bitcast(F32))
                nc.sync.dma_start(toks[gp][:], tkt[:])
                barriers[gp] = nc.gpsimd.collective_compute(
                    "AllReduce", mybir.AluOpType.add, replica_groups=PAIRS,
                    ins=[toks[gp]], outs=[toks2[gp]])

            # read peer partial chunk-by-chunk and add into gsb
            for dc in range(DC):
                gst = mpool.tile([P, 1, D], F32R, tag="gst")
                rd = nc.scalar.dma_start(
                    gst[:], Gsh[bass.ds(svp, 1), dc * P:(dc + 1) * P, :]
                    .rearrange("s (c p) n -> p (s c) n", p=P))
                add_dep_helper(rd.ins, barriers[dc // (DC // GPASS)].ins,
                               reason="peer G after pair barrier")
                nc.vector.tensor_add(gsb[:, dc], gsb[:, dc], gst[:, 0])

        # ------------- Phase V1 = G @ WvT, then A = PT^T @ V1 --------------
        with ExitStack() as actx:
            vpool = actx.enter_context(tc.tile_pool(name="vpool", bufs=1))
            apool = actx.enter_context(tc.tile_pool(name="apool", bufs=1))
            xtpool = actx.enter_context(tc.tile_pool(name="xtpool", bufs=4))
            opool = actx.enter_context(tc.tile_pool(name="opool", bufs=4))

            v1 = vpool.tile([P, DC, D], F32R, tag="v1")
            for dt in range(DC):
                for dh in range(2):
                    pv = psum.tile([P, FREE], F32, tag="ps")
                    for gc in range(DC):
                        # lhsT wants G[g, d-tile]; G is symmetric so the
                        # row-chunk gc doubles as the column chunk
                        nc.tensor.matmul(
                            pv[:], gsb[:, gc, dt * P:(dt + 1) * P],
                            wv[:, gc, dh * FREE:(dh + 1) * FREE],
                            start=(gc == 0), stop=(gc == DC - 1))
                    nc.vector.tensor_copy(
                        v1[:, dt, dh * FREE:(dh + 1) * FREE], pv[:])

            asb = apool.tile([P, DC, D], F32R, tag="asb")
            for et in range(DC):
                for dh in range(2):
                    pa = psum.tile([P, FREE], F32, tag="ps")
                    for dc in range(DC):
                        nc.tensor.matmul(
                            pa[:], pt[:, dc, et * P:(et + 1) * P],
                            v1[:, dc, dh * FREE:(dh + 1) * FREE],
                            start=(dc == 0), stop=(dc == DC - 1))
                    nc.vector.tensor_copy(
                        asb[:, et, dh * FREE:(dh + 1) * FREE], pa[:])

            # ------------- Phase out = x_half @ A, streamed per l-tile -----
            for lt in range(LH // P):
                xt = xtpool.tile([P, DC, P], F32R, tag="xt")
                nc.scalar.dma_start(
                    xt[:], chunked(xTh).bitcast(F32R)[:, :, lt * P:(lt + 1) * P])
                ot = opool.tile([P, D], F32, tag="ot")
                for dh in range(2):
                    po = psum.tile([P, FREE], F32, tag="ps")
                    for ec in range(DC):
                        nc.tensor.matmul(
                            po[:], xt[:, ec],
                            asb[:, ec, dh * FREE:(dh + 1) * FREE],
                            start=(ec == 0), stop=(ec == DC - 1))
                    nc.vector.tensor_copy(ot[:, dh * FREE:(dh + 1) * FREE],
                                          po[:])
                nc.sync.dma_start(out[lt * P:(lt + 1) * P, :], ot[:])

    nc.compile()
    return nc


_NC_CACHE = {}


def _get_nc():
    if "nc" not in _NC_CACHE:
        _NC_CACHE["nc"] = build_nc()
    return _NC_CACHE["nc"]


def run(inputs, trace=False):
    """Run the kernel on all 8 cores. Returns (full_output, BassKernelResults)."""
    x = np.asarray(inputs["x"], dtype=np.float32)
    Wq = np.asarray(inputs["Wq"], dtype=np.float32)
    Wk = np.asarray(inputs["Wk"], dtype=np.float32)
    Wv = np.asarray(inputs["Wv"], dtype=np.float32)

    inv_sqrt_d = np.float32(1.0 / np.sqrt(D))
    ptm = np.ascontiguousarray(Wk.T @ (Wq * inv_sqrt_d))  # PT[d,e]
    wvT = np.ascontiguousarray(Wv.T)

    in_maps = []
    for c in range(N_CORES):
        b, h = c // 2, c % 2
        xh = np.ascontiguousarray(x[b, h * LH:(h + 1) * LH, :])
        in_maps.append({
            "xb": xh,
            "xTh": np.ascontiguousarray(xh.T),
            "slots": np.array([[h, 1 - h]], dtype=np.uint32),
            "wvT": wvT, "ptm": ptm,
        })

    nc = _get_nc()
    res = run_bass_kernel_spmd(nc, in_maps, list(range(N_CORES)), trace=trace)

    full = np.empty((B, L, D), dtype=np.float32)
    for c in range(N_CORES):
        b, h = c // 2, c % 2
        full[b, h * LH:(h + 1) * LH, :] = res.results[c]["out"]
    return full, res


def kernel(**inputs):
    full, _ = run(inputs, trace=False)
    return full
